# revision 1
# baseline (speedup 1.0000x reference)
"""GAU denoising transformer forward pass on 8 Trainium2 NeuronCores.

Strategy: data-parallel over batch (B=16 -> 2 images per core). Each core
runs an identical NEFF on its own pair of images with all weights
replicated. Per core the residual stream is kept transposed in SBUF
(hT: H on partitions x 512 tokens = 2 images x 256 patches) in fp32 for
the whole 24-layer stack; weights are streamed in bf16 and all GEMMs run
in bf16 with fp32 PSUM accumulation.

Layout choices per layer:
  - uvqk GEMM is computed weight-stationary producing transposed outputs
    (feature on partitions) for u / q / k. The q/k columns are duplicated
    with swapped halves ("qswap"/"kswap" columns) so RoPE becomes three
    partition-aligned vector ops (no cross-partition reads).
  - v is computed activation-stationary producing the natural layout
    (token on partitions), which is what attn@v needs as stationary.
  - softmax over the free axis; attn (128x256 per l-chunk) transposed via
    the PE transpose; attn@v gives oT (feature on partitions); gating with
    uT; out-projection accumulates back into hT.
  - rmsnorm reduction over H (the partition axis) is done with a
    ones-vector matmul over squared activations; gnorm / fnorm_w are
    folded into the weight matrices on the host.
"""

import sys

for _p in ("/opt/trn_rl_repo",):
    if _p not in sys.path:
        sys.path.append(_p)

import numpy as np
import ml_dtypes

BF = ml_dtypes.bfloat16

IMG = 128
P = 8
H = 768
E = 1536
KD = 128          # key size
L = 256           # patches per image
PD = 192          # patch dim
NL = 24
B = 16
NCORES = 8
TOK = 512         # tokens per core (2 images x 256)
HC = H // 128     # 6 h-chunks
EC = E // 128     # 12 e-chunks
WUV_W = E + 2 * 128 + E   # permuted wuv width: u | q | k | v
V0 = E + 2 * 128          # column offset of v block


def _build(nl=NL, repeat=1):
    """Build + compile the Bass module. Returns nc."""
    import concourse.tile as tile
    from concourse import bacc, mybir
    from concourse.masks import make_identity

    F32 = mybir.dt.float32
    BF16 = mybir.dt.bfloat16
    AF = mybir.ActivationFunctionType

    nc = bacc.Bacc("TRN2", target_bir_lowering=False, debug=False,
                   num_devices=NCORES)

    d_xpt = nc.dram_tensor("xpt", [128, 2, TOK], BF16, kind="ExternalInput")
    d_temb = nc.dram_tensor("temb", [128, HC, 2], F32, kind="ExternalInput")
    d_pw = nc.dram_tensor("pw", [128, 2, H], BF16, kind="ExternalInput")
    d_wuv = nc.dram_tensor("wuv", [nl, 128, HC, WUV_W], BF16,
                           kind="ExternalInput")
    d_wout = nc.dram_tensor("wout", [nl, 128, EC, H], BF16,
                            kind="ExternalInput")
    d_upw = nc.dram_tensor("upw", [128, HC, PD], BF16, kind="ExternalInput")
    d_sperm = nc.dram_tensor("sperm", [128, 128], BF16, kind="ExternalInput")
    d_cq = nc.dram_tensor("cq", [128, TOK], BF16, kind="ExternalInput")
    d_sq = nc.dram_tensor("sq", [128, TOK], BF16, kind="ExternalInput")
    d_ck = nc.dram_tensor("ck", [128, TOK], BF16, kind="ExternalInput")
    d_sk = nc.dram_tensor("sk", [128, TOK], BF16, kind="ExternalInput")
    d_out = nc.dram_tensor("outt", [PD, TOK], F32, kind="ExternalOutput")

    from contextlib import ExitStack

    with tile.TileContext(nc) as tc, ExitStack() as ctx:
        pers = ctx.enter_context(tc.tile_pool(name="pers", bufs=1))
        wuvp = ctx.enter_context(tc.tile_pool(name="wuvp", bufs=2))
        woutp = ctx.enter_context(tc.tile_pool(name="woutp", bufs=1))
        rtmp = ctx.enter_context(tc.tile_pool(name="rtmp", bufs=1))
        hsqp = ctx.enter_context(tc.tile_pool(name="hsqp", bufs=7))
        utmp = ctx.enter_context(tc.tile_pool(name="utmp", bufs=3))
        attnp = ctx.enter_context(tc.tile_pool(name="attnp", bufs=3))
        attntp = ctx.enter_context(tc.tile_pool(name="attntp", bufs=3))
        statp = ctx.enter_context(tc.tile_pool(name="statp", bufs=4))
        rmsp = ctx.enter_context(tc.tile_pool(name="rmsp", bufs=1))
        rbp = ctx.enter_context(tc.tile_pool(name="rbp", bufs=2))

        psum = ctx.enter_context(tc.tile_pool(name="psum", bufs=1, space="PSUM"))

        # ---- persistent state + constants (per-chunk tiles) ----
        hT = [pers.tile([128, TOK], F32, name=f"hT{j}", tag=f"hT{j}")
              for j in range(HC)]
        hbf = [pers.tile([128, TOK], BF16, name=f"hbf{j}", tag=f"hbf{j}")
               for j in range(HC)]
        uT = [pers.tile([128, TOK], BF16, name=f"uT{e}", tag=f"uT{e}")
              for e in range(EC)]
        vn = [pers.tile([128, E], BF16, name=f"vn{t}", tag=f"vn{t}")
              for t in range(4)]
        ogT = [pers.tile([128, TOK], BF16, name=f"ogT{e}", tag=f"ogT{e}")
               for e in range(EC)]
        qp = pers.tile([128, TOK], BF16)           # roped q (scaled)
        kp = pers.tile([128, TOK], BF16)           # roped k
        cq = pers.tile([128, TOK], BF16)
        sq = pers.tile([128, TOK], BF16)
        ck = pers.tile([128, TOK], BF16)
        sk = pers.tile([128, TOK], BF16)
        temb = pers.tile([128, HC, 2], F32)
        xpt = pers.tile([128, 2, TOK], BF16)
        pw = pers.tile([128, 2, H], BF16)
        upw = pers.tile([128, HC, PD], BF16)
        ones = pers.tile([128, 1], BF16)
        ones1 = pers.tile([1, 1], F32)
        sperm = pers.tile([128, 128], BF16)
        qsb = pers.tile([128, TOK], BF16)
        ksb = pers.tile([128, TOK], BF16)
        ident = pers.tile([128, 128], BF16)
        identf = pers.tile([128, 128], F32)

        nc.sync.dma_start(cq, d_cq.ap())
        nc.sync.dma_start(sq, d_sq.ap())
        nc.sync.dma_start(ck, d_ck.ap())
        nc.sync.dma_start(sk, d_sk.ap())
        nc.sync.dma_start(temb, d_temb.ap())
        nc.sync.dma_start(xpt, d_xpt.ap())
        nc.sync.dma_start(pw, d_pw.ap())
        nc.sync.dma_start(upw, d_upw.ap())
        nc.sync.dma_start(sperm, d_sperm.ap())
        nc.vector.memset(ones, 1.0)
        nc.vector.memset(ones1, 1.0)
        make_identity(nc, ident)
        make_identity(nc, identf)

        # ---- patchify: hT = patch_W.T @ xp.T + temb ----
        for j in range(HC):
            ps = psum.tile([128, TOK], F32, tag=f"p{1 + j % 2}")
            for c in range(2):
                nc.tensor.matmul(ps, pw[:, c, j * 128:(j + 1) * 128],
                                 xpt[:, c, :], start=(c == 0), stop=(c == 1))
            for i in range(2):
                nc.vector.tensor_scalar_add(
                    hT[j][:, i * 256:(i + 1) * 256],
                    ps[:, i * 256:(i + 1) * 256],
                    temb[:, j, i:i + 1])
            nc.scalar.copy(hbf[j], hT[j])

        def make_squares(name):
            hsqs = []
            for j in range(HC):
                hsq = hsqp.tile([128, TOK], BF16, tag="hsq",
                                name=f"hsq_{name}{j}")
                nc.scalar.square(hsq, hT[j])
                hsqs.append(hsq)
            return hsqs

        hsqs_next = make_squares("init")

        def rms_stats(hsqs, name):
            """1/sqrt(mean(h^2)) per token: (1,TOK) rinv + (128,TOK) broadcast
            + (128,4) per-partition transposed copy. Newton rsqrt on DVE
            (eps=1e-6 after the sqrt is negligible vs bf16 noise)."""
            ss = psum.tile([1, TOK], F32, tag="p0", name=f"ss_{name}")
            for j in range(HC):
                nc.tensor.matmul(ss, ones, hsqs[j], start=(j == 0),
                                 stop=(j == HC - 1))
            # rsqrt via reciprocal-seeded Newton on a (128,4) layout (all
            # 128 DVE lanes; a (1,TOK) tile would serialize ~0.7us/op on one
            # lane). The (1,512)<->(128,4) shuffles are done with tiny PE
            # matmuls (K=1 scatter / M=1 gather) -- no DMA latency.
            ssb = rmsp.tile([1, TOK], F32, tag="ssb", name=f"ssb_{name}")
            nc.scalar.copy(ssb, ss)
            sst = psum.tile([128, 4], F32, tag="p7", name=f"sst_{name}")
            for t in range(4):
                nc.tensor.matmul(sst[:, t:t + 1], ssb[:, t * 128:(t + 1) * 128],
                                 ones1, start=True, stop=True)
            # m' = 16*mean(h^2) ~= 1.2 for this model; clamp to the seed's
            # convergence window; y = rsqrt(m')*4 = 1/rms (eps negligible).
            m = rmsp.tile([128, 4], F32, tag="m", name=f"m_{name}")
            nc.vector.tensor_scalar(m, sst, 16.0 / H, None,
                                    mybir.AluOpType.mult)
            nc.vector.tensor_scalar(m, m, 0.15, 6.0,
                                    mybir.AluOpType.max,
                                    mybir.AluOpType.min)
            rt = rbp.tile([128, 4], F32, tag="rt", name=f"rt_{name}")
            t1 = rmsp.tile([128, 4], F32, tag="t1", name=f"t1_{name}")
            nc.vector.reciprocal(rt, m)
            nc.vector.tensor_scalar(rt, rt, 0.5, 0.5,
                                    mybir.AluOpType.mult,
                                    mybir.AluOpType.add)
            for _ in range(4):   # newton: y *= 1.5 - 0.5*m*y^2
                nc.vector.tensor_mul(t1, rt, rt)
                nc.vector.tensor_mul(t1, t1, m)
                nc.vector.tensor_scalar(t1, t1, -0.5, 1.5,
                                        mybir.AluOpType.mult,
                                        mybir.AluOpType.add)
                nc.vector.tensor_mul(rt, rt, t1)
            nc.vector.tensor_scalar_mul(rt, rt, 4.0)
            row = psum.tile([1, TOK], F32, tag="p7", name=f"row_{name}")
            for t in range(4):
                nc.tensor.matmul(row[:, t * 128:(t + 1) * 128], rt[:, t:t + 1],
                                 identf, start=True, stop=True)
            yrow = rmsp.tile([1, TOK], F32, tag="yrow", name=f"yrow_{name}")
            nc.scalar.copy(yrow, row)
            rb = rbp.tile([128, TOK], F32, tag="rb", name=f"rb_{name}")
            nc.gpsimd.partition_broadcast(rb, yrow)
            return rb, rt

        for lrep in range(nl * repeat):
            li = lrep % nl
            wuv = wuvp.tile([128, HC, WUV_W], BF16, tag="wuv")
            nc.sync.dma_start(wuv, d_wuv.ap()[li])
            wout = woutp.tile([128, EC, H], BF16, tag="wout")
            nc.sync.dma_start(wout, d_wout.ap()[li])

            # norm stats off the critical path (uvqk GEMMs read raw hbf;
            # the 1/rms scale is applied at psum evacuation / in rope).
            # The squares were computed on ACT during the previous layer's
            # out-projection, so the sumsq matmuls can issue immediately.
            rb, rt = rms_stats(hsqs_next, f"l{lrep}")

            # ---- q/k col-tiles (weight-stationary, raw h); the swapped
            #      halves come from one permutation matmul each ----
            for t, sb in ((0, qsb), (1, ksb)):
                ct0 = E + t * 128
                ps = psum.tile([128, TOK], F32, tag=f"p{1 + t}")
                for j in range(HC):
                    nc.tensor.matmul(ps, wuv[:, j, ct0:ct0 + 128],
                                     hbf[j], start=(j == 0),
                                     stop=(j == HC - 1))
                nc.scalar.copy(sb, ps)
            qs_ps = psum.tile([128, TOK], F32, tag="p3")
            nc.tensor.matmul(qs_ps, sperm, qsb, start=True, stop=True)
            ks_ps = psum.tile([128, TOK], F32, tag="p4")
            nc.tensor.matmul(ks_ps, sperm, ksb, start=True, stop=True)

            # ---- rope (partition aligned), x 1/rms:
            #   q' = (Q*cos_q + Qswap*sins_q) * rb
            m1 = rtmp.tile([128, TOK], F32, tag="m1")
            m2 = rtmp.tile([128, TOK], F32, tag="m2")
            nc.vector.tensor_mul(m1, qsb, cq)
            nc.vector.tensor_mul(m2, qs_ps, sq)
            nc.vector.tensor_add(m1, m1, m2)
            nc.vector.tensor_mul(qp, m1, rb)
            m3 = rtmp.tile([128, TOK], F32, tag="m1")
            m4 = rtmp.tile([128, TOK], F32, tag="m2")
            nc.vector.tensor_mul(m3, ksb, ck)
            nc.vector.tensor_mul(m4, ks_ps, sk)
            nc.vector.tensor_add(m3, m3, m4)
            nc.vector.tensor_mul(kp, m3, rb)

            # ---- v natural (activation-stationary, raw h);
            #      silu(ps * rt) with per-partition (token) scale ----
            for tk in range(4):
                for ns in range(3):
                    ps = psum.tile([128, 512], F32, tag=f"p{5 + (tk * 3 + ns) % 2}")
                    for j in range(HC):
                        nc.tensor.matmul(
                            ps,
                            hbf[j][:, tk * 128:(tk + 1) * 128],
                            wuv[:, j, V0 + ns * 512:V0 + (ns + 1) * 512],
                            start=(j == 0), stop=(j == HC - 1))
                    nc.scalar.activation(vn[tk][:, ns * 512:(ns + 1) * 512],
                                         ps, AF.Silu, scale=rt[:, tk:tk + 1])

            # ---- scores + softmax (per image, per l-chunk) ----
            attn_sb = {}
            for i in range(2):
                for c in range(2):
                    sc = psum.tile([128, 256], F32, tag=("p7", "p0")[(i * 2 + c) % 2])
                    nc.tensor.matmul(sc, qp[:, i * 256 + c * 128:
                                            i * 256 + (c + 1) * 128],
                                     kp[:, i * 256:(i + 1) * 256],
                                     start=True, stop=True)
                    at = attnp.tile([128, 256], BF16, tag="attn")
                    sume = statp.tile([128, 1], F32, tag="sume")
                    nc.scalar.activation(at, sc, AF.Exp, scale=1.0,
                                         accum_out=sume)
                    rec = statp.tile([128, 1], F32, tag="rec")
                    nc.vector.reciprocal(rec, sume)
                    nc.vector.tensor_scalar_mul(at, at, rec)
                    attn_sb[(i, c)] = at

            # ---- u col-tiles (weight-stationary, raw h) ----
            for ct in range(EC):
                ps = psum.tile([128, TOK], F32, tag=f"p{1 + ct % 4}")
                for j in range(HC):
                    nc.tensor.matmul(ps, wuv[:, j, ct * 128:(ct + 1) * 128],
                                     hbf[j], start=(j == 0),
                                     stop=(j == HC - 1))
                ut = utmp.tile([128, TOK], BF16, tag="ut")
                nc.vector.tensor_mul(ut, ps, rb)
                nc.scalar.activation(uT[ct], ut, AF.Silu)

            # ---- transpose attn (PE) ----
            attnT = {}
            for i in range(2):
                for m in range(2):
                    aps = psum.tile([128, 256], BF16, tag=f"p{5 + (i * 2 + m) % 2}")
                    for c in range(2):
                        nc.tensor.transpose(
                            aps[:, c * 128:(c + 1) * 128],
                            attn_sb[(i, c)][:, m * 128:(m + 1) * 128],
                            ident)
                    asb = attntp.tile([128, 256], BF16, tag="ats")
                    nc.vector.tensor_copy(asb, aps)
                    attnT[(i, m)] = asb

            # ---- oT = (attn @ v).T ; gate with uT ----
            for i in range(2):
                for e in range(EC):
                    ops = psum.tile([128, 256], F32, tag=("p7", "p0")[(i * EC + e) % 2])
                    for m in range(2):
                        nc.tensor.matmul(ops,
                                         vn[i * 2 + m][:, e * 128:(e + 1) * 128],
                                         attnT[(i, m)],
                                         start=(m == 0), stop=(m == 1))
                    nc.vector.tensor_mul(ogT[e][:, i * 256:(i + 1) * 256],
                                         uT[e][:, i * 256:(i + 1) * 256],
                                         ops)

            # ---- out-projection + residual; squares for the next layer's
            #      rms + fresh raw-h bf16 copy, all per h-chunk ----
            hsqs_next = []
            for hp in range(HC):
                dps = psum.tile([128, TOK], F32, tag=f"p{1 + hp % 4}")
                for e in range(EC):
                    nc.tensor.matmul(dps, wout[:, e, hp * 128:(hp + 1) * 128],
                                     ogT[e], start=(e == 0),
                                     stop=(e == EC - 1))
                nc.vector.tensor_add(hT[hp], hT[hp], dps)
                hsq = hsqp.tile([128, TOK], BF16, tag="hsq",
                                name=f"hsq_l{lrep}_{hp}")
                nc.scalar.square(hsq, hT[hp])
                hsqs_next.append(hsq)
                nc.scalar.copy(hbf[hp], hT[hp])

        # ---- final norm + unpatch (fnorm_w folded into upw on host) ----
        rb, rt = rms_stats(hsqs_next, "fin")
        for j in range(HC):
            nc.vector.tensor_mul(hbf[j], hT[j], rb)
        for mchunk, msz in ((0, 128), (1, 64)):
            ps = psum.tile([128, TOK], F32, tag=f"p{5 + mchunk}")
            for j in range(HC):
                nc.tensor.matmul(ps[:msz, :],
                                 upw[:, j, mchunk * 128:mchunk * 128 + msz],
                                 hbf[j], start=(j == 0),
                                 stop=(j == HC - 1))
            osb = rtmp.tile([128, TOK], F32, tag="m1")
            nc.vector.tensor_copy(osb[:msz, :], ps[:msz, :])
            nc.sync.dma_start(d_out.ap()[mchunk * 128:mchunk * 128 + msz, :],
                              osb[:msz, :])

    nc.compile()
    return nc


_BUILD_CACHE = {}


def _get_nc(nl=NL, repeat=1):
    key = (nl, repeat)
    if key not in _BUILD_CACHE:
        _BUILD_CACHE[key] = _build(nl, repeat)
    return _BUILD_CACHE[key]


def _rope_tables():
    pos = np.arange(L)

    def sinemb(p, dim=64, base=1000.0):
        half = dim // 2
        freqs = np.exp(np.arange(half, dtype=np.float32)
                       * np.float32(-np.log(base) / (half - 1)))
        ang = p[:, None].astype(np.float32) * freqs[None, :]
        return np.concatenate([np.sin(ang), np.cos(ang)], axis=-1)

    w = IMG // P
    pe = np.concatenate([sinemb(pos // w), sinemb(pos % w)],
                        axis=-1).astype(np.float32)      # (256, 128)
    sinv = pe[:, :64].T                                  # (64, 256)
    cosv = pe[:, 64:].T
    COS = np.concatenate([cosv, cosv], axis=0)           # (128, 256)
    SINS = np.concatenate([-sinv, sinv], axis=0)
    COS2 = np.tile(COS, (1, 2))                          # (128, 512)
    SINS2 = np.tile(SINS, (1, 2))
    scale = np.float32(KD ** -0.5)
    return (np.ascontiguousarray(COS2 * scale).astype(BF),
            np.ascontiguousarray(SINS2 * scale).astype(BF),
            np.ascontiguousarray(COS2).astype(BF),
            np.ascontiguousarray(SINS2).astype(BF))


def _prep_weights(patch_W, t_emb, Wuv, Wout, gnorm, fnorm_w, unpatch_W, nl=NL):
    Wg = Wuv[:nl] * gnorm[:nl, :, None]                  # fold gnorm
    u = Wg[:, :, :E]
    q = Wg[:, :, 2 * E:2 * E + KD]
    k = Wg[:, :, 2 * E + KD:]
    v = Wg[:, :, E:2 * E]
    wuvp = np.concatenate([u, q, k, v], axis=2)          # (nl, 768, 3328)
    wuv_h = np.ascontiguousarray(
        wuvp.reshape(nl, HC, 128, WUV_W).transpose(0, 2, 1, 3)).astype(BF)
    wout_h = np.ascontiguousarray(
        Wout[:nl].reshape(nl, EC, 128, H).transpose(0, 2, 1, 3)).astype(BF)
    pw_pad = np.zeros((256, H), np.float32)
    pw_pad[:PD] = patch_W
    pw_h = np.ascontiguousarray(
        pw_pad.reshape(2, 128, H).transpose(1, 0, 2)).astype(BF)
    upw = fnorm_w[:, None] * unpatch_W                   # fold fnorm
    upw_h = np.ascontiguousarray(
        upw.reshape(HC, 128, PD).transpose(1, 0, 2)).astype(BF)
    return wuv_h, wout_h, pw_h, upw_h


def _patchify(xc):
    """(2,3,128,128) -> (512, 192) token-major patches."""
    g = IMG // P
    xp = xc.reshape(2, 3, g, P, g, P).transpose(0, 2, 4, 3, 5, 1)
    return np.ascontiguousarray(xp.reshape(2 * L, PD))


def _unpatchify(oT):
    """(192, 512) -> (2, 3, 128, 128)."""
    g = IMG // P
    out = np.empty((2, 3, IMG, IMG), np.float32)
    for i in range(2):
        h = oT[:, i * L:(i + 1) * L].T                   # (256, 192)
        out[i] = (h.reshape(g, g, P, P, 3)
                  .transpose(4, 0, 2, 1, 3).reshape(3, IMG, IMG))
    return out


def make_in_maps(x, t_idx, patch_W, t_emb, Wuv, Wout, gnorm, fnorm_w,
                 unpatch_W, nl=NL):
    x = np.asarray(x, np.float32)
    t_idx = np.asarray(t_idx).astype(np.int64)
    patch_W = np.asarray(patch_W, np.float32)
    t_emb = np.asarray(t_emb, np.float32)
    Wuv = np.asarray(Wuv, np.float32)
    Wout = np.asarray(Wout, np.float32)
    gnorm = np.asarray(gnorm, np.float32)
    fnorm_w = np.asarray(fnorm_w, np.float32)
    unpatch_W = np.asarray(unpatch_W, np.float32)

    wuv_h, wout_h, pw_h, upw_h = _prep_weights(
        patch_W, t_emb, Wuv, Wout, gnorm, fnorm_w, unpatch_W, nl)
    cqt, sqt, ckt, skt = _rope_tables()
    sperm = np.ascontiguousarray(np.roll(np.eye(128, dtype=np.float32),
                                         64, axis=0)).astype(BF)

    in_maps = []
    for c in range(NCORES):
        xc = x[2 * c:2 * c + 2]
        xp = _patchify(xc)                               # (512, 192)
        xpad = np.zeros((TOK, 256), np.float32)
        xpad[:, :PD] = xp
        xpt = np.ascontiguousarray(
            xpad.T.reshape(2, 128, TOK).transpose(1, 0, 2)).astype(BF)
        te = t_emb[t_idx[2 * c:2 * c + 2, 0]]            # (2, 768)
        tembT = np.ascontiguousarray(
            te.T.reshape(HC, 128, 2).transpose(1, 0, 2)).astype(np.float32)
        in_maps.append({
            "xpt": xpt, "temb": tembT, "pw": pw_h, "wuv": wuv_h,
            "wout": wout_h, "upw": upw_h, "cq": cqt, "sq": sqt,
            "ck": ckt, "sk": skt, "sperm": sperm,
        })
    return in_maps


def kernel(**inputs):
    from concourse.bass_utils import run_bass_kernel_spmd

    nc = _get_nc()
    in_maps = make_in_maps(**inputs)
    res = run_bass_kernel_spmd(nc, in_maps, core_ids=list(range(NCORES)))
    out = np.empty((B, 3, IMG, IMG), np.float32)
    for c in range(NCORES):
        out[2 * c:2 * c + 2] = _unpatchify(res.results[c]["outt"])
    return out

